# revision 46
# baseline (speedup 1.0000x reference)
# Multi-head causal attention (B=4, S=2048, D=1024, H=16) on 8 TRN2 NeuronCores.
#
# Sharding: batch x query-chunk. Core c handles batch b=c//2 and two 512-row
# query chunks of that batch: cores with c%2==0 take real chunks (0, 3),
# c%2==1 take (1, 2). The SPMD program is identical on every core: two query
# "slots" with fixed kk-tile capacities (8, 16); per-core causality/padding
# is expressed via multiplicative masks in input data.
#
# v3 structure: Q proj + K/V first-half run up front; K/V second-half
# projections are emitted as PE "filler" groups interleaved into the cap-8
# attention phase (their outputs are first needed at t>=8 of the cap-16
# phase). Output-projection units are interleaved at block boundaries of the
# cap-16 phase. y is written in bf16 and converted on the host.
#
#   St[kk, q]: Kt[d, s], Qt[d, q]; St = Kt_tile.T @ Qt (2 heads packed into
#   one 2-bank PSUM tile, exp'd in a single ACT op)
#   P = exp(St) * mask
#   OT[dv, q] += V_aug[kk, 65].T @ P  -- V carries a ones column, so PSUM
#     row 64 accumulates the softmax denominators for free.
import sys

if '/opt/trn_rl_repo' not in sys.path:
    sys.path.insert(0, '/opt/trn_rl_repo')

import numpy as np

B, S, D = 4, 2048, 1024
H, DK = 16, 64
NCORES = 8
SC = 512
NKT = S // 128            # 16 kk tiles
HPN = D // 128            # 8 head-pairs
CAPS = (8, 16)            # kk-tile capacity per slot (uniform across cores)
CHUNKS = [(0, 3), (1, 2)]  # real chunk pair per core parity

_CACHE = {}


def _build_program():
    import contextlib

    import concourse.tile as tile
    from concourse import bacc, mybir

    F32 = mybir.dt.float32
    BF16 = mybir.dt.bfloat16
    EXP = mybir.ActivationFunctionType.Exp

    nc = bacc.Bacc("TRN2", target_bir_lowering=False, debug=False,
                   num_devices=NCORES)

    xT_d = nc.dram_tensor("xT", [D, S], BF16, kind="ExternalInput")
    xQT_d = nc.dram_tensor("xQT", [D, 2 * SC], BF16, kind="ExternalInput")
    wqT_d = nc.dram_tensor("wqT", [D, D], BF16, kind="ExternalInput")
    wkT_d = nc.dram_tensor("wkT", [D, D], BF16, kind="ExternalInput")
    wvT_d = nc.dram_tensor("wvT", [D, D], BF16, kind="ExternalInput")
    woT_d = nc.dram_tensor("woT", [D, D], BF16, kind="ExternalInput")
    bias_d = nc.dram_tensor("bias", [1, D], BF16, kind="ExternalInput")
    masks_d = nc.dram_tensor("masks", [128, 16 * 512], BF16,
                             kind="ExternalInput")
    y_d = nc.dram_tensor("y", [2 * SC, D], BF16, kind="ExternalOutput")

    with tile.TileContext(nc) as tc, contextlib.ExitStack() as ctx:
        smalls = ctx.enter_context(tc.tile_pool(name="smalls", bufs=1))
        p_OT = ctx.enter_context(tc.tile_pool(name="otp", bufs=1))
        p_Kt = ctx.enter_context(tc.tile_pool(name="ktp", bufs=1))
        p_Qt = ctx.enter_context(tc.tile_pool(name="qtp", bufs=1))
        p_V = ctx.enter_context(tc.tile_pool(name="vp", bufs=1))
        p_mk = ctx.enter_context(tc.tile_pool(name="mk", bufs=1))
        # closed manually after the cap-8 phase to free 48 KB/partition
        proj_ctx = contextlib.ExitStack()
        p_xh1 = proj_ctx.enter_context(tc.tile_pool(name="xth1", bufs=1))
        p_wv = proj_ctx.enter_context(tc.tile_pool(name="wfv", bufs=1))
        p_wk = proj_ctx.enter_context(tc.tile_pool(name="wfk", bufs=1))

        OT = p_OT.tile([128, HPN * 2 * SC], BF16, tag="OT")
        Kt = p_Kt.tile([128, HPN * S], BF16, tag="Kt")
        Qt = p_Qt.tile([128, HPN * 2 * SC], BF16, tag="Qt")
        Vsb = p_V.tile([128, NKT * H * 65], BF16, tag="Vsb")
        masks_sb = p_mk.tile([128, 16 * 512], BF16, tag="masks")
        bias_sb = smalls.tile([1, D], BF16, tag="bias")
        ones1f = smalls.tile([1, 128], F32, tag="ones1f")
        nc.vector.memset(ones1f[:], 1.0)
        ones1 = smalls.tile([1, 128], BF16, tag="ones1")
        nc.vector.tensor_copy(ones1[:], ones1f[:])
        ones256f = smalls.tile([128, 256], F32, tag="ones256f")
        nc.vector.memset(ones256f[:], 1.0)

        # ones columns of V_aug (all 16 s-tiles, one strided copy)
        nc.vector.tensor_copy(
            Vsb[:].rearrange("p (s h c) -> p s h c", s=NKT, c=65)
            [:, :, :, 64:65],
            ones256f[:].rearrange("p (s h) -> p s h", s=NKT)[:, :, :, None])

        wv = p_wv.tile([128, 8 * D], BF16, tag="wv")
        wk = p_wk.tile([128, 8 * D], BF16, tag="wk")
        xh1 = p_xh1.tile([128, 8 * 1024], BF16, tag="xh1")
        xhs = [None, xh1]

        # ------- preamble: Q projection, then K/V first half -------------
        # xh0/wq/xq close with this scope, freeing SBUF for attention pools
        with tc.tile_pool(name="xth0", bufs=1) as p_xh0, \
             tc.tile_pool(name="wf2", bufs=1) as p_w2, \
             tc.tile_pool(name="xqs", bufs=8) as p_xq, \
             tc.tile_pool(name="psq", bufs=8, space="PSUM") as psq:
            xhs[0] = p_xh0.tile([128, 8 * 1024], BF16, tag="xh0",
                                name="xh0")
            wq = p_w2.tile([128, 8 * D], BF16, tag="w2")
            # ci-major order: a ci=1 load never queues ahead of a ci=0 load
            # it transitively depends on (buf reuse + in-order DMA queue)
            xq_tiles = {}
            for ci in range(2):
                for k in range(8):
                    if ci == 0:
                        nc.sync.dma_start(
                            wq[:, k * D:(k + 1) * D],
                            wqT_d.ap()[k * 128:(k + 1) * 128, :])
                    xq1 = p_xq.tile([128, 512], BF16, tag="xq",
                                    name=f"xq_{ci}_{k}")
                    nc.sync.dma_start(
                        xq1[:],
                        xQT_d.ap()[k * 128:(k + 1) * 128,
                                   ci * SC:(ci + 1) * SC])
                    xq_tiles[(ci, k)] = xq1
            # K/V inputs: first half of x + wk right behind, wv next,
            # second half afterwards; masks/bias late on the gpsimd queue.
            for k in range(8):
                nc.sync.dma_start(
                    xhs[0][:, k * 1024:(k + 1) * 1024],
                    xT_d.ap()[k * 128:(k + 1) * 128, 0:1024])
                nc.sync.dma_start(
                    wk[:, k * D:(k + 1) * D],
                    wkT_d.ap()[k * 128:(k + 1) * 128, :])
            for k in range(8):
                nc.sync.dma_start(
                    wv[:, k * D:(k + 1) * D],
                    wvT_d.ap()[k * 128:(k + 1) * 128, :])
                nc.sync.dma_start(
                    xhs[1][:, k * 1024:(k + 1) * 1024],
                    xT_d.ap()[k * 128:(k + 1) * 128, 1024:2048])
            nc.gpsimd.dma_start(masks_sb[:], masks_d.ap())
            nc.gpsimd.dma_start(bias_sb[:], bias_d.ap())

            for ci in range(2):
                ps8 = [psq.tile([128, 512], F32, tag="ps",
                                name=f"psq_{ci}_{hp}") for hp in range(HPN)]
                for k in range(8):
                    for hp in range(HPN):
                        nc.tensor.matmul(
                            ps8[hp][:],
                            wq[:, k * D + hp * 128:k * D + (hp + 1) * 128],
                            xq_tiles[(ci, k)][:],
                            start=(k == 0), stop=(k == 7))
                for hp in range(HPN):
                    nc.vector.tensor_copy(
                        Qt[:, hp * 2 * SC + ci * SC:
                           hp * 2 * SC + (ci + 1) * SC],
                        ps8[hp][:])

            # --------- K + V projections for the first sequence half -----
            xh = xhs[0]
            for sc2 in range(2):
                ps8 = [psq.tile([128, 512], F32, tag="ps",
                                name=f"psk_{sc2}_{hp}")
                       for hp in range(HPN)]
                for k in range(8):
                    for hp in range(HPN):
                        nc.tensor.matmul(
                            ps8[hp][:],
                            wk[:, k * D + hp * 128:k * D + (hp + 1) * 128],
                            xh[:, k * 1024 + sc2 * 512:
                               k * 1024 + (sc2 + 1) * 512],
                            start=(k == 0), stop=(k == 7))
                for hp in range(HPN):
                    nc.vector.tensor_copy(
                        Kt[:, hp * S + sc2 * 512:hp * S + (sc2 + 1) * 512],
                        ps8[hp][:])
            for sti in range(8):
                for dvc in range(2):
                    ps = psq.tile([128, 512], F32, tag="ps")
                    for k in range(8):
                        nc.tensor.matmul(
                            ps[:],
                            xh[:, k * 1024 + sti * 128:
                               k * 1024 + (sti + 1) * 128],
                            wv[:, k * D + dvc * 512:k * D + (dvc + 1) * 512],
                            start=(k == 0), stop=(k == 7))
                    off = sti * 1040 + dvc * 520
                    nc.vector.tensor_copy(
                        Vsb[:, off:off + 520]
                        .rearrange("p (h c) -> p h c", c=65)[:, :, 0:64],
                        ps[:].rearrange("p (h c) -> p h c", c=64))

        # ---------------- attention with interleaved fillers -------------
        # (rs/bc/P pools are per-phase so pool closes stay LIFO-ordered)
        with contextlib.nullcontext():

            # ---- filler generators: K/V projections for sequence half 1,
            # each a closure emitting ~1.8us of PE work into pool `fx`.
            def kh1_filler(sc, hp, fx):
                def emit():
                    xh = xhs[1]
                    sc2 = sc - 2
                    ps = fx.tile([128, 512], F32, tag="fx",
                                 name=f"fk_{sc}_{hp}")
                    for k in range(8):
                        nc.tensor.matmul(
                            ps[:],
                            wk[:, k * D + hp * 128:k * D + (hp + 1) * 128],
                            xh[:, k * 1024 + sc2 * 512:
                               k * 1024 + (sc2 + 1) * 512],
                            start=(k == 0), stop=(k == 7))
                    nc.vector.tensor_copy(
                        Kt[:, hp * S + sc * 512:hp * S + (sc + 1) * 512],
                        ps[:])
                return emit

            def vh1_filler(sti, dvc, fx):
                def emit():
                    xh = xhs[1]
                    st_g = 8 + sti
                    ps = fx.tile([128, 512], F32, tag="fx",
                                 name=f"fv_{sti}_{dvc}")
                    for k in range(8):
                        nc.tensor.matmul(
                            ps[:],
                            xh[:, k * 1024 + sti * 128:
                               k * 1024 + (sti + 1) * 128],
                            wv[:, k * D + dvc * 512:k * D + (dvc + 1) * 512],
                            start=(k == 0), stop=(k == 7))
                    off = st_g * 1040 + dvc * 520
                    nc.vector.tensor_copy(
                        Vsb[:, off:off + 520]
                        .rearrange("p (h c) -> p h c", c=65)[:, :, 0:64],
                        ps[:].rearrange("p (h c) -> p h c", c=64))
                return emit

            def outproj_unit(qi, nc2, pool, wo, p_yb):
                def emit():
                    ps = pool.tile([128, 512], F32, tag="av",
                                   name=f"psy_{qi}_{nc2}")
                    for dc in range(8):
                        nc.tensor.matmul(
                            ps[:],
                            OT[:, dc * 2 * SC + qi * 128:
                               dc * 2 * SC + (qi + 1) * 128],
                            wo[:, dc * D + nc2 * 512:
                               dc * D + (nc2 + 1) * 512],
                            start=(dc == 0), stop=False)
                    nc.tensor.matmul(
                        ps[:], ones1[:],
                        bias_sb[0:1, nc2 * 512:(nc2 + 1) * 512],
                        start=False, stop=True)
                    yb = p_yb.tile([128, 512], BF16, tag="yb")
                    nc.vector.tensor_copy(yb[:], ps[:])
                    nc.sync.dma_start(
                        y_d.ap()[qi * 128:(qi + 1) * 128,
                                 nc2 * 512:(nc2 + 1) * 512], yb[:])
                return emit

            def attn_block(ci, bl, cap, p_st, p_av, pump,
                           p_P, p_rs, p_bc, bc_psum):
                av = [p_av.tile([128, 512], F32, tag="av",
                                name=f"av_{ci}_{bl}_{i}")
                      for i in range(4)]

                def emit_av(t, p_tiles):
                    for hp_i in range(2):
                        for hh in range(2):
                            hi = 2 * hp_i + hh
                            off = (t * 1040 + (2 * bl + hp_i) * 130 +
                                   hh * 65)
                            nc.tensor.matmul(
                                av[hi][0:65, :],
                                Vsb[:, off:off + 65],
                                p_tiles[hp_i][:, hh * 512:(hh + 1) * 512],
                                start=(t == 0), stop=(t == cap - 1))

                def emit_scores_exp(t, hp_i, p_cur):
                    hp = 2 * bl + hp_i
                    st = p_st.tile([128, 1024], F32, tag="st")
                    for hh in range(2):
                        r0 = 64 * hh
                        nc.tensor.matmul(
                            st[:, hh * 512:(hh + 1) * 512],
                            Kt[r0:r0 + 64,
                               hp * S + t * 128:hp * S + (t + 1) * 128],
                            Qt[r0:r0 + 64,
                               hp * 2 * SC + ci * SC:
                               hp * 2 * SC + (ci + 1) * SC],
                            start=True, stop=True,
                            tile_position=(r0, 0))
                    p1 = p_P.tile([128, 1024], BF16, tag="p")
                    nc.scalar.activation(p1[:], st[:], EXP)
                    if ci == 0 or t >= 8:
                        midx = t if ci == 0 else 8 + (t - 8)
                        p2 = p_P.tile([128, 1024], BF16, tag="p")
                        for hf in range(2):
                            nc.vector.tensor_mul(
                                p2[:, hf * 512:(hf + 1) * 512],
                                p1[:, hf * 512:(hf + 1) * 512],
                                masks_sb[:, midx * 512:(midx + 1) * 512])
                        p1 = p2
                    p_cur.append(p1)

                # lag-2 software pipeline; AV + filler PE work sits between
                # the two score groups so the st-pool WAR dependency on
                # exp(hp_i=0) never stalls the PE queue head.
                pending = []
                for t in range(cap):
                    p_cur = []
                    emit_scores_exp(t, 0, p_cur)
                    if len(pending) > 2:
                        tt, pp_t = pending.pop(0)
                        emit_av(tt, pp_t)
                    emit_scores_exp(t, 1, p_cur)
                    # filler AFTER hp_i=1: covers the st-pool WAR wait on
                    # exp(t, hp_i=1) before scores(t+1, hp_i=0) issues
                    if pump:
                        pump(t)
                    pending.append((t, p_cur))
                for tt, pp_t in pending:
                    emit_av(tt, pp_t)
                # normalize, one head-pair at a time: denominators broadcast
                # across partitions via a rank-1 PE matmul (ones^T @ rs) --
                # ~4x faster than gpsimd partition_broadcast and off the
                # DVE critical path
                for hp_i in range(2):
                    hp = 2 * bl + hp_i
                    rs = p_rs.tile([1, 1024], F32, tag="rs")
                    for hh in range(2):
                        hi = 2 * hp_i + hh
                        nc.vector.tensor_copy(
                            rs[0:1, hh * 512:hh * 512 + 512],
                            av[hi][64:65, :])
                    bcs = bc_psum(hp_i)
                    for hh in range(2):
                        nc.tensor.matmul(
                            bcs[hh], ones1f[:],
                            rs[0:1, hh * 512:(hh + 1) * 512],
                            start=True, stop=True)
                    rbc = p_bc.tile([128, 1024], F32, tag="rbc")
                    scr = p_bc.tile([128, 1024], F32, tag="scr")
                    for hh in range(2):
                        nc.vector.reciprocal_approx_accurate(
                            rbc[:, hh * 512:(hh + 1) * 512], bcs[hh],
                            scratch=scr[:, hh * 512:(hh + 1) * 512])
                    for hh in range(2):
                        hi = 2 * hp_i + hh
                        r0 = 64 * hh
                        nc.vector.tensor_mul(
                            OT[r0:r0 + 64,
                               hp * 2 * SC + ci * SC:
                               hp * 2 * SC + (ci + 1) * SC],
                            av[hi][0:64, :],
                            rbc[r0:r0 + 64, hh * 512:hh * 512 + 512])
                # boundary PE work keeps the tensor engine (and HAM) warm
                # while the norm chain drains on DVE/GpSimd
                if pump:
                    pump(None)

            # ---- cap-8 phase: st x1 + av x4 + flex x2 banks; K/V second
            # half drains through the flex pool as filler work.
            with tc.tile_pool(name="rs0", bufs=1) as p_rs0, \
                 tc.tile_pool(name="bcp0", bufs=1) as p_bc0, \
                 tc.tile_pool(name="pp0", bufs=8) as p_P0, \
                 tc.tile_pool(name="pst0", bufs=1, space="PSUM") as p_st0, \
                 tc.tile_pool(name="pav0", bufs=4, space="PSUM") as p_av0, \
                 tc.tile_pool(name="pfx", bufs=2, space="PSUM") as p_fx:
                # cap-8 fillers: K/V projections for kk tiles 8-15 (needed
                # from cap-16 t=8). One filler per t-iteration = 32 slots.
                q8 = []
                for hp in range(HPN):
                    q8.append(kh1_filler(2, hp, p_fx))
                for sti in range(4):
                    for dvc in range(2):
                        q8.append(vh1_filler(sti, dvc, p_fx))
                for hp in range(HPN):
                    q8.append(kh1_filler(3, hp, p_fx))
                for sti in range(4, 8):
                    for dvc in range(2):
                        q8.append(vh1_filler(sti, dvc, p_fx))

                def pump8(t):
                    if t is not None and q8:
                        q8.pop(0)()

                bcn = [0]

                def bc_psum8(hp_i):
                    bcn[0] += 1
                    out = []
                    for hh in range(2):
                        fxt = p_fx.tile([128, 512], F32, tag="fx",
                                        name=f"bc8_{bcn[0]}_{hh}")
                        out.append(fxt[:])
                    return out

                for bl in range(HPN // 2):
                    attn_block(0, bl, CAPS[0], p_st0, p_av0, pump8,
                               p_P0, p_rs0, p_bc0, bc_psum8)
                while q8:
                    q8.pop(0)()

            proj_ctx.close()  # free xh1/wv/wk SBUF for wo/yb

            # ---- cap-16 phase: st x1 + av x4 + flex x2; K/V for kk tiles
            # 12-15 drain through bl0's early t-slots, out-proj spreads
            # through bl1..bl3.
            with tc.tile_pool(name="wo", bufs=1) as p_wo, \
                 tc.tile_pool(name="ybp", bufs=4) as p_yb, \
                 tc.tile_pool(name="rs1", bufs=1) as p_rs1, \
                 tc.tile_pool(name="bcp1", bufs=1) as p_bc1, \
                 tc.tile_pool(name="pp1", bufs=8) as p_P1, \
                 tc.tile_pool(name="pst1", bufs=2, space="PSUM") as p_st1, \
                 tc.tile_pool(name="pav1", bufs=4, space="PSUM") as p_av1:
                wo = p_wo.tile([128, 8 * D], BF16, tag="wo")
                for k in range(8):
                    nc.sync.dma_start(
                        wo[:, k * D:(k + 1) * D],
                        woT_d.ap()[k * 128:(k + 1) * 128, :])
                outp = [outproj_unit(qi, nc2, p_av1, wo, p_yb)
                        for qi in range(4) for nc2 in range(2)]
                state = {'bl': 0}

                def pump16(t):
                    # av-pool steal is only safe between blocks
                    if t is None:
                        for _ in range(2):
                            if outp:
                                outp.pop(0)()

                bcn1 = [0]

                def bc_psum16(hp_i):
                    bcn1[0] += 1
                    bct = p_st1.tile([128, 1024], F32, tag="st",
                                     name=f"bc16_{bcn1[0]}")
                    return [bct[:, 0:512], bct[:, 512:1024]]

                for bl in range(HPN // 2):
                    state['bl'] = bl
                    attn_block(1, bl, CAPS[1], p_st1, p_av1, pump16,
                               p_P1, p_rs1, p_bc1, bc_psum16)
                while outp:
                    outp.pop(0)()
                # remaining out-proj: the cap-16 slot's queries
                for qi in range(4, 8):
                    for nc2 in range(2):
                        outproj_unit(qi, nc2, p_av1, wo, p_yb)()

    nc.compile()
    return nc


def _get_program():
    if 'nc' not in _CACHE:
        _CACHE['nc'] = _build_program()
    return _CACHE['nc']


def _tri_masks():
    p = np.arange(128)[:, None]
    f = np.arange(SC)[None, :]
    return [(p <= f - 128 * r).astype(np.float32) for r in range(4)]


def _masks_for_core(c):
    """Multiplicative masks [128, 16*512]: slot t<8 serves the cap-8 slot
    (chunk j1), t>=8 the cap-16 slot (chunk j2, applied at t>=8 only)."""
    import ml_dtypes
    tri = _tri_masks()
    ones = np.ones((128, SC), np.float32)
    zeros = np.zeros((128, SC), np.float32)
    j_pair = CHUNKS[c % 2]
    out = np.zeros((128, 16 * 512), np.float32)
    for ci, cap in enumerate(CAPS):
        j = j_pair[ci]
        t0 = 0 if ci == 0 else 8
        for t in range(t0, cap):
            if t < 4 * j:
                m = ones
            elif t < 4 * j + 4:
                m = tri[t - 4 * j]
            else:
                m = zeros
            out[:, t * 512:(t + 1) * 512] = m
    return out.astype(ml_dtypes.bfloat16)


def kernel(x, w_q, w_k, w_v, w_o, b_o):
    import ml_dtypes
    from concourse.bass_utils import run_bass_kernel_spmd

    BF = ml_dtypes.bfloat16
    x = np.asarray(x, dtype=np.float32)
    nc = _get_program()

    scale = np.float32(1.0 / np.sqrt(DK))
    common = {
        "wqT": np.ascontiguousarray(
            (np.asarray(w_q, np.float32).T * scale)).astype(BF),
        "wkT": np.ascontiguousarray(np.asarray(w_k, np.float32).T).astype(BF),
        "wvT": np.ascontiguousarray(np.asarray(w_v, np.float32).T).astype(BF),
        "woT": np.ascontiguousarray(np.asarray(w_o, np.float32).T).astype(BF),
        "bias": np.asarray(b_o, np.float32)[None, :].astype(BF),
    }

    in_maps = []
    for c in range(NCORES):
        b = c // 2
        j1, j2 = CHUNKS[c % 2]
        xb = x[b]
        xq = np.concatenate(
            [xb[j1 * SC:(j1 + 1) * SC], xb[j2 * SC:(j2 + 1) * SC]], axis=0)
        in_maps.append({
            "xT": np.ascontiguousarray(xb.T).astype(BF),
            "xQT": np.ascontiguousarray(xq.T).astype(BF),
            "masks": _masks_for_core(c),
            **common,
        })

    res = run_bass_kernel_spmd(nc, in_maps, core_ids=list(range(NCORES)),
                               trace=_CACHE.get('trace', False),
                               tmpdir=_CACHE.get('tmpdir'))
    _CACHE['last_res'] = res

    y = np.empty((B, S, D), dtype=np.float32)
    for c in range(NCORES):
        b = c // 2
        j1, j2 = CHUNKS[c % 2]
        yc = np.asarray(res.results[c]["y"], dtype=np.float32)
        y[b, j1 * SC:(j1 + 1) * SC] = yc[0:SC]
        y[b, j2 * SC:(j2 + 1) * SC] = yc[SC:2 * SC]
    return y


# revision 48
# speedup vs baseline: 1.0404x; 1.0404x over previous
# Multi-head causal attention (B=4, S=2048, D=1024, H=16) on 8 TRN2 NeuronCores.
#
# Sharding: batch x query-chunk. Core c handles batch b=c//2 and two 512-row
# query chunks of that batch: cores with c%2==0 take real chunks (0, 3),
# c%2==1 take (1, 2). The SPMD program is identical on every core: two query
# "slots" with fixed kk-tile capacities (8, 16); per-core causality/padding
# is expressed via multiplicative masks in input data.
#
# v3 structure: Q proj + K/V first-half run up front; K/V second-half
# projections are emitted as PE "filler" groups interleaved into the cap-8
# attention phase (their outputs are first needed at t>=8 of the cap-16
# phase). Output-projection units are interleaved at block boundaries of the
# cap-16 phase. y is written in bf16 and converted on the host.
#
#   St[kk, q]: Kt[d, s], Qt[d, q]; St = Kt_tile.T @ Qt (2 heads packed into
#   one 2-bank PSUM tile, exp'd in a single ACT op)
#   P = exp(St) * mask
#   OT[dv, q] += V_aug[kk, 65].T @ P  -- V carries a ones column, so PSUM
#     row 64 accumulates the softmax denominators for free.
import sys

if '/opt/trn_rl_repo' not in sys.path:
    sys.path.insert(0, '/opt/trn_rl_repo')

import numpy as np

B, S, D = 4, 2048, 1024
H, DK = 16, 64
NCORES = 8
SC = 512
NKT = S // 128            # 16 kk tiles
HPN = D // 128            # 8 head-pairs
CAPS = (8, 16)            # kk-tile capacity per slot (uniform across cores)
CHUNKS = [(0, 3), (1, 2)]  # real chunk pair per core parity

_CACHE = {}


def _build_program():
    import contextlib

    import concourse.tile as tile
    from concourse import bacc, mybir

    F32 = mybir.dt.float32
    BF16 = mybir.dt.bfloat16
    EXP = mybir.ActivationFunctionType.Exp

    nc = bacc.Bacc("TRN2", target_bir_lowering=False, debug=False,
                   num_devices=NCORES)

    xT_d = nc.dram_tensor("xT", [D, S], BF16, kind="ExternalInput")
    xQT_d = nc.dram_tensor("xQT", [D, 2 * SC], BF16, kind="ExternalInput")
    wqT_d = nc.dram_tensor("wqT", [D, D], BF16, kind="ExternalInput")
    wkT_d = nc.dram_tensor("wkT", [D, D], BF16, kind="ExternalInput")
    wvT_d = nc.dram_tensor("wvT", [D, D], BF16, kind="ExternalInput")
    woT_d = nc.dram_tensor("woT", [D, D], BF16, kind="ExternalInput")
    bias_d = nc.dram_tensor("bias", [1, D], BF16, kind="ExternalInput")
    masks_d = nc.dram_tensor("masks", [128, 16 * 512], BF16,
                             kind="ExternalInput")
    y_d = nc.dram_tensor("y", [2 * SC, D], BF16, kind="ExternalOutput")

    with tile.TileContext(nc) as tc, contextlib.ExitStack() as ctx:
        smalls = ctx.enter_context(tc.tile_pool(name="smalls", bufs=1))
        p_OT = ctx.enter_context(tc.tile_pool(name="otp", bufs=1))
        p_Kt = ctx.enter_context(tc.tile_pool(name="ktp", bufs=1))
        p_Qt = ctx.enter_context(tc.tile_pool(name="qtp", bufs=1))
        p_V = ctx.enter_context(tc.tile_pool(name="vp", bufs=1))
        p_mk = ctx.enter_context(tc.tile_pool(name="mk", bufs=1))
        # closed manually after the cap-8 phase to free 48 KB/partition
        proj_ctx = contextlib.ExitStack()
        p_xh1 = proj_ctx.enter_context(tc.tile_pool(name="xth1", bufs=1))
        p_wv = proj_ctx.enter_context(tc.tile_pool(name="wfv", bufs=1))
        p_wk = proj_ctx.enter_context(tc.tile_pool(name="wfk", bufs=1))

        OT = p_OT.tile([128, HPN * 2 * SC], BF16, tag="OT")
        Kt = p_Kt.tile([128, HPN * S], BF16, tag="Kt")
        Qt = p_Qt.tile([128, HPN * 2 * SC], BF16, tag="Qt")
        Vsb = p_V.tile([128, NKT * H * 65], BF16, tag="Vsb")
        masks_sb = p_mk.tile([128, 16 * 512], BF16, tag="masks")
        bias_sb = smalls.tile([1, D], BF16, tag="bias")
        ones1f = smalls.tile([1, 128], F32, tag="ones1f")
        nc.vector.memset(ones1f[:], 1.0)
        ones1 = smalls.tile([1, 128], BF16, tag="ones1")
        nc.vector.tensor_copy(ones1[:], ones1f[:])
        ones256f = smalls.tile([128, 256], F32, tag="ones256f")
        nc.vector.memset(ones256f[:], 1.0)

        # ones columns of V_aug (all 16 s-tiles, one strided copy)
        nc.vector.tensor_copy(
            Vsb[:].rearrange("p (s h c) -> p s h c", s=NKT, c=65)
            [:, :, :, 64:65],
            ones256f[:].rearrange("p (s h) -> p s h", s=NKT)[:, :, :, None])

        wv = p_wv.tile([128, 8 * D], BF16, tag="wv")
        wk = p_wk.tile([128, 8 * D], BF16, tag="wk")
        xh1 = p_xh1.tile([128, 8 * 1024], BF16, tag="xh1")
        xhs = [None, xh1]

        # ------- preamble: Q projection, then K/V first half -------------
        # xh0/wq/xq close with this scope, freeing SBUF for attention pools
        with tc.tile_pool(name="xth0", bufs=1) as p_xh0, \
             tc.tile_pool(name="wf2", bufs=1) as p_w2, \
             tc.tile_pool(name="xqs", bufs=8) as p_xq, \
             tc.tile_pool(name="psq", bufs=8, space="PSUM") as psq:
            xhs[0] = p_xh0.tile([128, 8 * 1024], BF16, tag="xh0",
                                name="xh0")
            wq = p_w2.tile([128, 8 * D], BF16, tag="w2")
            # ci-major order: a ci=1 load never queues ahead of a ci=0 load
            # it transitively depends on (buf reuse + in-order DMA queue)
            xq_tiles = {}
            for ci in range(2):
                for k in range(8):
                    if ci == 0:
                        nc.sync.dma_start(
                            wq[:, k * D:(k + 1) * D],
                            wqT_d.ap()[k * 128:(k + 1) * 128, :])
                    xq1 = p_xq.tile([128, 512], BF16, tag="xq",
                                    name=f"xq_{ci}_{k}")
                    nc.sync.dma_start(
                        xq1[:],
                        xQT_d.ap()[k * 128:(k + 1) * 128,
                                   ci * SC:(ci + 1) * SC])
                    xq_tiles[(ci, k)] = xq1
            # K/V inputs: first half of x + wk right behind, wv next,
            # second half afterwards; masks/bias late on the gpsimd queue.
            for k in range(8):
                nc.sync.dma_start(
                    xhs[0][:, k * 1024:(k + 1) * 1024],
                    xT_d.ap()[k * 128:(k + 1) * 128, 0:1024])
                nc.sync.dma_start(
                    wk[:, k * D:(k + 1) * D],
                    wkT_d.ap()[k * 128:(k + 1) * 128, :])
            for k in range(8):
                nc.sync.dma_start(
                    wv[:, k * D:(k + 1) * D],
                    wvT_d.ap()[k * 128:(k + 1) * 128, :])
                nc.sync.dma_start(
                    xhs[1][:, k * 1024:(k + 1) * 1024],
                    xT_d.ap()[k * 128:(k + 1) * 128, 1024:2048])
            nc.gpsimd.dma_start(masks_sb[:], masks_d.ap())
            nc.gpsimd.dma_start(bias_sb[:], bias_d.ap())

            for ci in range(2):
                ps8 = [psq.tile([128, 512], F32, tag="ps",
                                name=f"psq_{ci}_{hp}") for hp in range(HPN)]
                for k in range(8):
                    for hp in range(HPN):
                        nc.tensor.matmul(
                            ps8[hp][:],
                            wq[:, k * D + hp * 128:k * D + (hp + 1) * 128],
                            xq_tiles[(ci, k)][:],
                            start=(k == 0), stop=(k == 7))
                for hp in range(HPN):
                    nc.vector.tensor_copy(
                        Qt[:, hp * 2 * SC + ci * SC:
                           hp * 2 * SC + (ci + 1) * SC],
                        ps8[hp][:])

            # --------- K + V projections for the first sequence half -----
            xh = xhs[0]
            for sc2 in range(2):
                ps8 = [psq.tile([128, 512], F32, tag="ps",
                                name=f"psk_{sc2}_{hp}")
                       for hp in range(HPN)]
                for k in range(8):
                    for hp in range(HPN):
                        nc.tensor.matmul(
                            ps8[hp][:],
                            wk[:, k * D + hp * 128:k * D + (hp + 1) * 128],
                            xh[:, k * 1024 + sc2 * 512:
                               k * 1024 + (sc2 + 1) * 512],
                            start=(k == 0), stop=(k == 7))
                for hp in range(HPN):
                    nc.vector.tensor_copy(
                        Kt[:, hp * S + sc2 * 512:hp * S + (sc2 + 1) * 512],
                        ps8[hp][:])
            for sti in range(8):
                for dvc in range(2):
                    ps = psq.tile([128, 512], F32, tag="ps")
                    for k in range(8):
                        nc.tensor.matmul(
                            ps[:],
                            xh[:, k * 1024 + sti * 128:
                               k * 1024 + (sti + 1) * 128],
                            wv[:, k * D + dvc * 512:k * D + (dvc + 1) * 512],
                            start=(k == 0), stop=(k == 7))
                    off = sti * 1040 + dvc * 520
                    nc.vector.tensor_copy(
                        Vsb[:, off:off + 520]
                        .rearrange("p (h c) -> p h c", c=65)[:, :, 0:64],
                        ps[:].rearrange("p (h c) -> p h c", c=64))

        # ---------------- attention with interleaved fillers -------------
        # (rs/bc/P pools are per-phase so pool closes stay LIFO-ordered)
        with contextlib.nullcontext():

            # ---- filler generators: K/V projections for sequence half 1,
            # each a closure emitting ~1.8us of PE work into pool `fx`.
            def kh1_filler(sc, hp, fx):
                def emit():
                    xh = xhs[1]
                    sc2 = sc - 2
                    ps = fx.tile([128, 512], F32, tag="fx",
                                 name=f"fk_{sc}_{hp}")
                    for k in range(8):
                        nc.tensor.matmul(
                            ps[:],
                            wk[:, k * D + hp * 128:k * D + (hp + 1) * 128],
                            xh[:, k * 1024 + sc2 * 512:
                               k * 1024 + (sc2 + 1) * 512],
                            start=(k == 0), stop=(k == 7))
                    nc.vector.tensor_copy(
                        Kt[:, hp * S + sc * 512:hp * S + (sc + 1) * 512],
                        ps[:])
                return emit

            def vh1_filler(sti, dvc, fx):
                def emit():
                    xh = xhs[1]
                    st_g = 8 + sti
                    ps = fx.tile([128, 512], F32, tag="fx",
                                 name=f"fv_{sti}_{dvc}")
                    for k in range(8):
                        nc.tensor.matmul(
                            ps[:],
                            xh[:, k * 1024 + sti * 128:
                               k * 1024 + (sti + 1) * 128],
                            wv[:, k * D + dvc * 512:k * D + (dvc + 1) * 512],
                            start=(k == 0), stop=(k == 7))
                    off = st_g * 1040 + dvc * 520
                    nc.vector.tensor_copy(
                        Vsb[:, off:off + 520]
                        .rearrange("p (h c) -> p h c", c=65)[:, :, 0:64],
                        ps[:].rearrange("p (h c) -> p h c", c=64))
                return emit

            def outproj_unit(qi, nc2, pool, wo, p_yb):
                def emit():
                    ps = pool.tile([128, 512], F32, tag="av",
                                   name=f"psy_{qi}_{nc2}")
                    for dc in range(8):
                        nc.tensor.matmul(
                            ps[:],
                            OT[:, dc * 2 * SC + qi * 128:
                               dc * 2 * SC + (qi + 1) * 128],
                            wo[:, dc * D + nc2 * 512:
                               dc * D + (nc2 + 1) * 512],
                            start=(dc == 0), stop=False)
                    nc.tensor.matmul(
                        ps[:], ones1[:],
                        bias_sb[0:1, nc2 * 512:(nc2 + 1) * 512],
                        start=False, stop=True)
                    yb = p_yb.tile([128, 512], BF16, tag="yb")
                    nc.vector.tensor_copy(yb[:], ps[:])
                    nc.sync.dma_start(
                        y_d.ap()[qi * 128:(qi + 1) * 128,
                                 nc2 * 512:(nc2 + 1) * 512], yb[:])
                return emit

            def attn_block(ci, bl, cap, p_st, p_av, pump,
                           p_P, p_rs, p_bc, bc_psum):
                av = [p_av.tile([128, 512], F32, tag="av",
                                name=f"av_{ci}_{bl}_{i}")
                      for i in range(4)]

                def emit_av(t, p_tiles):
                    for hp_i in range(2):
                        for hh in range(2):
                            hi = 2 * hp_i + hh
                            off = (t * 1040 + (2 * bl + hp_i) * 130 +
                                   hh * 65)
                            nc.tensor.matmul(
                                av[hi][0:65, :],
                                Vsb[:, off:off + 65],
                                p_tiles[hp_i][:, hh * 512:(hh + 1) * 512],
                                start=(t == 0), stop=(t == cap - 1))

                def emit_scores_exp(t, hp_i, p_cur):
                    hp = 2 * bl + hp_i
                    st = p_st.tile([128, 1024], F32, tag="st")
                    for hh in range(2):
                        r0 = 64 * hh
                        nc.tensor.matmul(
                            st[:, hh * 512:(hh + 1) * 512],
                            Kt[r0:r0 + 64,
                               hp * S + t * 128:hp * S + (t + 1) * 128],
                            Qt[r0:r0 + 64,
                               hp * 2 * SC + ci * SC:
                               hp * 2 * SC + (ci + 1) * SC],
                            start=True, stop=True,
                            tile_position=(r0, 0))
                    p1 = p_P.tile([128, 1024], BF16, tag="p")
                    nc.scalar.activation(p1[:], st[:], EXP)
                    if ci == 0 or t >= 8:
                        midx = t if ci == 0 else 8 + (t - 8)
                        p2 = p_P.tile([128, 1024], BF16, tag="p")
                        for hf in range(2):
                            nc.vector.tensor_mul(
                                p2[:, hf * 512:(hf + 1) * 512],
                                p1[:, hf * 512:(hf + 1) * 512],
                                masks_sb[:, midx * 512:(midx + 1) * 512])
                        p1 = p2
                    p_cur.append(p1)

                # lag-2 software pipeline; AV + filler PE work sits between
                # the two score groups so the st-pool WAR dependency on
                # exp(hp_i=0) never stalls the PE queue head.
                pending = []
                for t in range(cap):
                    p_cur = []
                    emit_scores_exp(t, 0, p_cur)
                    if len(pending) > 2:
                        tt, pp_t = pending.pop(0)
                        emit_av(tt, pp_t)
                    emit_scores_exp(t, 1, p_cur)
                    # filler AFTER hp_i=1: covers the st-pool WAR wait on
                    # exp(t, hp_i=1) before scores(t+1, hp_i=0) issues
                    if pump:
                        pump(t)
                    pending.append((t, p_cur))
                for tt, pp_t in pending:
                    emit_av(tt, pp_t)
                # normalize, one head-pair at a time: denominators broadcast
                # across partitions via a rank-1 PE matmul (ones^T @ rs) --
                # ~4x faster than gpsimd partition_broadcast and off the
                # DVE critical path
                for hp_i in range(2):
                    hp = 2 * bl + hp_i
                    rs = p_rs.tile([1, 1024], F32, tag="rs")
                    for hh in range(2):
                        hi = 2 * hp_i + hh
                        nc.vector.tensor_copy(
                            rs[0:1, hh * 512:hh * 512 + 512],
                            av[hi][64:65, :])
                    rbc = p_bc.tile([128, 1024], F32, tag="rbc")
                    scr = p_bc.tile([128, 1024], F32, tag="scr")
                    if bc_psum is not None:
                        bcs = bc_psum(hp_i)
                        for hh in range(2):
                            nc.tensor.matmul(
                                bcs[hh], ones1f[:],
                                rs[0:1, hh * 512:(hh + 1) * 512],
                                start=True, stop=True)
                        for hh in range(2):
                            nc.vector.reciprocal_approx_accurate(
                                rbc[:, hh * 512:(hh + 1) * 512], bcs[hh],
                                scratch=scr[:, hh * 512:(hh + 1) * 512])
                    else:
                        bc = p_bc.tile([128, 1024], F32, tag="bc")
                        nc.gpsimd.partition_broadcast(bc[:], rs[:])
                        nc.vector.reciprocal_approx_accurate(
                            rbc[:], bc[:], scratch=scr[:])
                    for hh in range(2):
                        hi = 2 * hp_i + hh
                        r0 = 64 * hh
                        nc.vector.tensor_mul(
                            OT[r0:r0 + 64,
                               hp * 2 * SC + ci * SC:
                               hp * 2 * SC + (ci + 1) * SC],
                            av[hi][0:64, :],
                            rbc[r0:r0 + 64, hh * 512:hh * 512 + 512])
                # boundary PE work keeps the tensor engine (and HAM) warm
                # while the norm chain drains on DVE/GpSimd
                if pump:
                    pump(None)

            # ---- cap-8 phase: st x1 + av x4 + flex x2 banks; K/V second
            # half drains through the flex pool as filler work.
            with tc.tile_pool(name="rs0", bufs=1) as p_rs0, \
                 tc.tile_pool(name="bcp0", bufs=1) as p_bc0, \
                 tc.tile_pool(name="pp0", bufs=8) as p_P0, \
                 tc.tile_pool(name="pst0", bufs=1, space="PSUM") as p_st0, \
                 tc.tile_pool(name="pav0", bufs=4, space="PSUM") as p_av0, \
                 tc.tile_pool(name="pfx", bufs=2, space="PSUM") as p_fx:
                # cap-8 fillers: K/V projections for kk tiles 8-15 (needed
                # from cap-16 t=8). One filler per t-iteration = 32 slots.
                q8 = []
                for hp in range(HPN):
                    q8.append(kh1_filler(2, hp, p_fx))
                for sti in range(4):
                    for dvc in range(2):
                        q8.append(vh1_filler(sti, dvc, p_fx))
                for hp in range(HPN):
                    q8.append(kh1_filler(3, hp, p_fx))
                for sti in range(4, 8):
                    for dvc in range(2):
                        q8.append(vh1_filler(sti, dvc, p_fx))

                def pump8(t):
                    if t is not None and q8:
                        q8.pop(0)()

                bcn = [0]

                def bc_psum8(hp_i):
                    bcn[0] += 1
                    out = []
                    for hh in range(2):
                        fxt = p_fx.tile([128, 512], F32, tag="fx",
                                        name=f"bc8_{bcn[0]}_{hh}")
                        out.append(fxt[:])
                    return out

                for bl in range(HPN // 2):
                    attn_block(0, bl, CAPS[0], p_st0, p_av0, pump8,
                               p_P0, p_rs0, p_bc0, bc_psum8)
                while q8:
                    q8.pop(0)()

            proj_ctx.close()  # free xh1/wv/wk SBUF for wo/yb

            # ---- cap-16 phase: st x1 + av x4 + flex x2; K/V for kk tiles
            # 12-15 drain through bl0's early t-slots, out-proj spreads
            # through bl1..bl3.
            with tc.tile_pool(name="wo", bufs=1) as p_wo, \
                 tc.tile_pool(name="ybp", bufs=4) as p_yb, \
                 tc.tile_pool(name="rs1", bufs=1) as p_rs1, \
                 tc.tile_pool(name="bcp1", bufs=1) as p_bc1, \
                 tc.tile_pool(name="pp1", bufs=8) as p_P1, \
                 tc.tile_pool(name="pst1", bufs=2, space="PSUM") as p_st1, \
                 tc.tile_pool(name="pav1", bufs=4, space="PSUM") as p_av1:
                wo = p_wo.tile([128, 8 * D], BF16, tag="wo")
                for k in range(8):
                    nc.sync.dma_start(
                        wo[:, k * D:(k + 1) * D],
                        woT_d.ap()[k * 128:(k + 1) * 128, :])
                outp = [outproj_unit(qi, nc2, p_av1, wo, p_yb)
                        for qi in range(4) for nc2 in range(2)]
                state = {'bl': 0}

                def pump16(t):
                    # av-pool steal is only safe between blocks
                    if t is None:
                        for _ in range(2):
                            if outp:
                                outp.pop(0)()

                for bl in range(HPN // 2):
                    state['bl'] = bl
                    attn_block(1, bl, CAPS[1], p_st1, p_av1, pump16,
                               p_P1, p_rs1, p_bc1, None)
                while outp:
                    outp.pop(0)()
                # remaining out-proj: the cap-16 slot's queries
                for qi in range(4, 8):
                    for nc2 in range(2):
                        outproj_unit(qi, nc2, p_av1, wo, p_yb)()

    nc.compile()
    return nc


def _get_program():
    if 'nc' not in _CACHE:
        _CACHE['nc'] = _build_program()
    return _CACHE['nc']


def _tri_masks():
    p = np.arange(128)[:, None]
    f = np.arange(SC)[None, :]
    return [(p <= f - 128 * r).astype(np.float32) for r in range(4)]


def _masks_for_core(c):
    """Multiplicative masks [128, 16*512]: slot t<8 serves the cap-8 slot
    (chunk j1), t>=8 the cap-16 slot (chunk j2, applied at t>=8 only)."""
    import ml_dtypes
    tri = _tri_masks()
    ones = np.ones((128, SC), np.float32)
    zeros = np.zeros((128, SC), np.float32)
    j_pair = CHUNKS[c % 2]
    out = np.zeros((128, 16 * 512), np.float32)
    for ci, cap in enumerate(CAPS):
        j = j_pair[ci]
        t0 = 0 if ci == 0 else 8
        for t in range(t0, cap):
            if t < 4 * j:
                m = ones
            elif t < 4 * j + 4:
                m = tri[t - 4 * j]
            else:
                m = zeros
            out[:, t * 512:(t + 1) * 512] = m
    return out.astype(ml_dtypes.bfloat16)


def kernel(x, w_q, w_k, w_v, w_o, b_o):
    import ml_dtypes
    from concourse.bass_utils import run_bass_kernel_spmd

    BF = ml_dtypes.bfloat16
    x = np.asarray(x, dtype=np.float32)
    nc = _get_program()

    scale = np.float32(1.0 / np.sqrt(DK))
    common = {
        "wqT": np.ascontiguousarray(
            (np.asarray(w_q, np.float32).T * scale)).astype(BF),
        "wkT": np.ascontiguousarray(np.asarray(w_k, np.float32).T).astype(BF),
        "wvT": np.ascontiguousarray(np.asarray(w_v, np.float32).T).astype(BF),
        "woT": np.ascontiguousarray(np.asarray(w_o, np.float32).T).astype(BF),
        "bias": np.asarray(b_o, np.float32)[None, :].astype(BF),
    }

    in_maps = []
    for c in range(NCORES):
        b = c // 2
        j1, j2 = CHUNKS[c % 2]
        xb = x[b]
        xq = np.concatenate(
            [xb[j1 * SC:(j1 + 1) * SC], xb[j2 * SC:(j2 + 1) * SC]], axis=0)
        in_maps.append({
            "xT": np.ascontiguousarray(xb.T).astype(BF),
            "xQT": np.ascontiguousarray(xq.T).astype(BF),
            "masks": _masks_for_core(c),
            **common,
        })

    res = run_bass_kernel_spmd(nc, in_maps, core_ids=list(range(NCORES)),
                               trace=_CACHE.get('trace', False),
                               tmpdir=_CACHE.get('tmpdir'))
    _CACHE['last_res'] = res

    y = np.empty((B, S, D), dtype=np.float32)
    for c in range(NCORES):
        b = c // 2
        j1, j2 = CHUNKS[c % 2]
        yc = np.asarray(res.results[c]["y"], dtype=np.float32)
        y[b, j1 * SC:(j1 + 1) * SC] = yc[0:SC]
        y[b, j2 * SC:(j2 + 1) * SC] = yc[SC:2 * SC]
    return y
